# revision 3
# baseline (speedup 1.0000x reference)
"""CoLightAgent forward kernel for 8 Trainium2 NeuronCores.

Math note: in the reference, ne = broadcast(adj @ emb) over the agent axis i,
so nh.sum(axis=3) / hid.sum(axis=3) are independent of i and collapse to
per-batch vectors S_n, S_h of shape [T].  The final gather keeps only row
tgt[b] of the agent branch.  The whole [B,N,N,T] intermediate disappears:

    E    = relu(relu(obs @ We1 + be1) @ We2 + be2)        # [N, T] per batch
    AE   = adj @ E                                        # [N, T]
    S_n  = sum_j relu(AE @ Wn + bn)[j, :]                 # [T]
    S_h  = sum_j relu(AE @ Wh + bh)[j, :]                 # [T]
    a    = relu(E[tgt] @ Wl + bl)                         # [T]
    attn = softmax_d((a * S_n).reshape(D, H).T)           # [H, D]
    g    = mean_h(attn * S_h.reshape(D, H).T)             # [D]
    act  = g @ Wa + ba                                    # [ACT]

Sharding: data-parallel over the batch; core c computes batch c % 4 in full
(cores 4..7 duplicate 0..3 and their outputs are ignored).  All matmuls keep
the contraction dim on partitions; activations flow as
    E1T [t, n] -> E [n, t'] -> AET [t, m]
so every stage can feed the next as lhsT/rhs without any transposes.
"""

import numpy as np

import concourse.bacc as bacc
import concourse.mybir as mybir
import concourse.tile as tile
from concourse import bass_utils
from concourse.bass import ts

B, N, OBS, ACT = 4, 256, 40, 8
HEAD, DIM = 8, 32
T = HEAD * DIM
P = 128
F32 = mybir.dt.float32
AF = mybir.ActivationFunctionType
AX = mybir.AxisListType

_CACHE = {}


def _build_nc():
    nc = bacc.Bacc("TRN2", target_bir_lowering=False, debug=False, num_devices=8)

    def din(name, shape):
        return nc.dram_tensor(name, list(shape), F32, kind="ExternalInput")

    d_obsT = din("obsT", [OBS, N])
    d_adjT = din("adjT", [N, N])
    d_We1 = din("We1", [OBS, T])
    d_We2 = din("We2", [T, T])
    d_Wl = din("Wl", [T, T])
    d_Wn = din("Wn", [T, T])
    d_Wh = din("Wh", [T, T])
    d_Wa = din("Wa", [DIM, ACT])
    d_be1 = din("be1", [T, 1])
    d_be2f = din("be2f", [P, T])  # be2 row replicated across partitions
    d_bl = din("bl", [T, 1])
    d_bn = din("bn", [T, 1])
    d_bh = din("bh", [T, 1])
    d_ba = din("ba", [ACT, 1])
    d_onehot = din("onehot", [N, 1])

    d_out = nc.dram_tensor("act_out", [ACT], F32, kind="ExternalOutput")
    d_scr = nc.dram_tensor("shuffle_scr", [4 * P], F32)

    with tile.TileContext(nc) as tc:
        with (
            tc.tile_pool(name="w", bufs=1) as wp,
            tc.tile_pool(name="work", bufs=2) as work,
            tc.tile_pool(name="mmps", bufs=3, space="PSUM") as ps,
            tc.tile_pool(name="vecps", bufs=2, space="PSUM") as psv,
            tc.tile_pool(name="smps", bufs=1, space="PSUM") as pss,
        ):
            # ---- load weights / inputs ------------------------------------
            obsT_t = wp.tile([P, N], F32)
            We1_t = wp.tile([P, T], F32)
            nc.any.memset(obsT_t[:], 0.0)
            nc.any.memset(We1_t[:], 0.0)
            nc.sync.dma_start(obsT_t[:OBS, :], d_obsT.ap())
            nc.sync.dma_start(We1_t[:OBS, :], d_We1.ap())

            adjT_t = wp.tile([P, 2, N], F32)
            We2_t = wp.tile([P, 2, T], F32)
            Wl_t = wp.tile([P, 2, T], F32)
            Wn_t = wp.tile([P, 2, T], F32)
            Wh_t = wp.tile([P, 2, T], F32)
            for d_m, t_m in ((d_adjT, adjT_t), (d_We2, We2_t), (d_Wl, Wl_t),
                             (d_Wn, Wn_t), (d_Wh, Wh_t)):
                nc.sync.dma_start(t_m[:], d_m.ap().rearrange("(s p) m -> p s m", p=P))

            be1_t = wp.tile([P, 2, 1], F32)
            bl_t = wp.tile([P, 2, 1], F32)
            bn_t = wp.tile([P, 2, 1], F32)
            bh_t = wp.tile([P, 2, 1], F32)
            oh_t = wp.tile([P, 2, 1], F32)
            for d_m, t_m in ((d_be1, be1_t), (d_bl, bl_t), (d_bn, bn_t),
                             (d_bh, bh_t), (d_onehot, oh_t)):
                nc.sync.dma_start(t_m[:], d_m.ap().rearrange("(s p) o -> p s o", p=P))

            be2f_t = wp.tile([P, T], F32)
            nc.sync.dma_start(be2f_t[:], d_be2f.ap())
            Wa_t = wp.tile([P, ACT], F32)
            nc.any.memset(Wa_t[:], 0.0)
            nc.sync.dma_start(Wa_t[:DIM, :], d_Wa.ap())
            ba_t = wp.tile([ACT, 1], F32)
            nc.sync.dma_start(ba_t[:], d_ba.ap())
            ones_t = wp.tile([P, 1], F32)
            nc.any.memset(ones_t[:], 1.0)

            # ---- stage 1: E1T[t, n] = relu(We1.T @ obsT + be1) ------------
            E1T_t = wp.tile([P, 2, N], F32)
            for s in range(2):
                pm = ps.tile([P, N], F32, tag="mm")
                nc.tensor.matmul(pm[:], We1_t[:, ts(s, P)], obsT_t[:],
                                 start=True, stop=True)
                nc.scalar.activation(E1T_t[:, s, :], pm[:], AF.Relu,
                                     bias=be1_t[:, s, :])

            # ---- stage 2: E[n, t'] = relu(E1 @ We2 + be2) -----------------
            E_t = wp.tile([P, 2, T], F32)
            for s in range(2):
                pm = ps.tile([P, T], F32, tag="mm")
                nc.tensor.matmul(pm[:], E1T_t[:, 0, ts(s, P)], We2_t[:, 0, :],
                                 start=True, stop=False)
                nc.tensor.matmul(pm[:], E1T_t[:, 1, ts(s, P)], We2_t[:, 1, :],
                                 start=False, stop=True)
                nc.vector.tensor_add(E_t[:, s, :], pm[:], be2f_t[:])
                nc.vector.tensor_scalar_max(E_t[:, s, :], E_t[:, s, :], 0.0)

            # ---- stage 3: AET[t, m] = (adj @ E).T = E-as-lhsT @ adjT ------
            AET_t = wp.tile([P, 2, N], F32)
            for s in range(2):
                pm = ps.tile([P, N], F32, tag="mm")
                nc.tensor.matmul(pm[:], E_t[:, 0, ts(s, P)], adjT_t[:, 0, :],
                                 start=True, stop=False)
                nc.tensor.matmul(pm[:], E_t[:, 1, ts(s, P)], adjT_t[:, 1, :],
                                 start=False, stop=True)
                nc.vector.tensor_copy(AET_t[:, s, :], pm[:])

            # ---- stages 4/5: S_n / S_h (relu + row-sum fused) -------------
            Sn_t = wp.tile([P, 2, 1], F32)
            Sh_t = wp.tile([P, 2, 1], F32)
            for W_t, b_t, S_t in ((Wn_t, bn_t, Sn_t), (Wh_t, bh_t, Sh_t)):
                for s in range(2):
                    pm = ps.tile([P, N], F32, tag="mm")
                    nc.tensor.matmul(pm[:], W_t[:, 0, ts(s, P)], AET_t[:, 0, :],
                                     start=True, stop=False)
                    nc.tensor.matmul(pm[:], W_t[:, 1, ts(s, P)], AET_t[:, 1, :],
                                     start=False, stop=True)
                    zt = work.tile([P, N], F32, tag="zsc")
                    nc.scalar.activation(zt[:], pm[:], AF.Relu,
                                         bias=b_t[:, s, :],
                                         accum_out=S_t[:, s, :])

            # ---- stage 6: e_t = E[tgt, :] via one-hot ---------------------
            et_t = wp.tile([P, 2, 1], F32)
            for s in range(2):
                pv = psv.tile([P, 1], F32, tag="vec")
                nc.tensor.matmul(pv[:], E_t[:, 0, ts(s, P)], oh_t[:, 0, :],
                                 start=True, stop=False)
                nc.tensor.matmul(pv[:], E_t[:, 1, ts(s, P)], oh_t[:, 1, :],
                                 start=False, stop=True)
                nc.vector.tensor_copy(et_t[:, s, :], pv[:])

            # ---- stage 7: a = relu(Wl.T @ e_t + bl) -----------------------
            a_t = wp.tile([P, 2, 1], F32)
            for s in range(2):
                pv = psv.tile([P, 1], F32, tag="vec")
                nc.tensor.matmul(pv[:], Wl_t[:, 0, ts(s, P)], et_t[:, 0, :],
                                 start=True, stop=False)
                nc.tensor.matmul(pv[:], Wl_t[:, 1, ts(s, P)], et_t[:, 1, :],
                                 start=False, stop=True)
                nc.scalar.activation(a_t[:, s, :], pv[:], AF.Relu,
                                     bias=bl_t[:, s, :])

            # ---- stage 8: bounce l = a*S_n and S_h into [H, D] layout -----
            LS_t = wp.tile([P, 4], F32)
            nc.vector.tensor_mul(LS_t[:, 0:2], a_t[:, :, 0], Sn_t[:, :, 0])
            nc.vector.tensor_copy(LS_t[:, 2:4], Sh_t[:, :, 0])
            nc.sync.dma_start(d_scr.ap().rearrange("(c p) -> p c", p=P), LS_t[:])
            lsh8_t = wp.tile([HEAD, 2, DIM], F32)
            nc.sync.dma_start(
                lsh8_t[:], d_scr.ap().rearrange("(c d h) -> h c d", h=HEAD, c=2))
            l8 = lsh8_t[:, 0, :]
            sh8 = lsh8_t[:, 1, :]

            # ---- stage 9: attn = softmax_d(l), out8 = attn * S_h ----------
            mx_t = wp.tile([HEAD, 1], F32)
            nmx_t = wp.tile([HEAD, 1], F32)
            p8_t = wp.tile([HEAD, DIM], F32)
            ssum_t = wp.tile([HEAD, 1], F32)
            rs_t = wp.tile([HEAD, 1], F32)
            out8_t = wp.tile([P, DIM], F32)
            nc.any.memset(out8_t[:], 0.0)
            nc.vector.reduce_max(mx_t[:], l8, axis=AX.X)
            nc.scalar.mul(nmx_t[:], mx_t[:], -1.0)
            nc.scalar.activation(p8_t[:], l8, AF.Exp, bias=nmx_t[:],
                                 accum_out=ssum_t[:])
            nc.vector.reciprocal(rs_t[:], ssum_t[:])
            nc.vector.scalar_tensor_tensor(out8_t[:HEAD, :], p8_t[:], rs_t[:],
                                           sh8, mybir.AluOpType.mult,
                                           mybir.AluOpType.mult)

            # ---- stage 10: g = mean_h(out8);  act = g @ Wa + ba -----------
            g_t = wp.tile([P, 1], F32)
            nc.any.memset(g_t[:], 0.0)
            pg = pss.tile([DIM, 1], F32, tag="g")
            nc.tensor.matmul(pg[:], out8_t[:], ones_t[:], start=True, stop=True)
            nc.vector.tensor_copy(g_t[:DIM, :], pg[:])
            pa = pss.tile([ACT, 1], F32, tag="acct")
            nc.tensor.matmul(pa[:], Wa_t[:], g_t[:], start=True, stop=True)
            res_t = wp.tile([ACT, 1], F32)
            nc.scalar.activation(res_t[:], pa[:], AF.Identity, bias=ba_t[:],
                                 scale=1.0 / HEAD)
            nc.sync.dma_start(d_out.ap(), res_t[:, 0])

    nc.compile()
    return nc


def get_nc():
    if "nc" not in _CACHE:
        _CACHE["nc"] = _build_nc()
    return _CACHE["nc"]


def make_in_maps(x, adj, We1, be1, We2, be2, Wl, bl, Wn, bn, Wh, bh, Wa, ba):
    f = lambda v: np.ascontiguousarray(np.asarray(v, np.float32))
    x = f(x)
    tgt = x[:, -1, 0].astype(np.int32)
    obs = x[:, :-1, :]
    shared = {
        "adjT": f(np.asarray(adj).T),
        "We1": f(We1), "We2": f(We2), "Wl": f(Wl), "Wn": f(Wn), "Wh": f(Wh),
        "Wa": f(Wa),
        "be1": f(be1).reshape(T, 1),
        "be2f": f(np.broadcast_to(np.asarray(be2, np.float32), (P, T))),
        "bl": f(bl).reshape(T, 1),
        "bn": f(bn).reshape(T, 1),
        "bh": f(bh).reshape(T, 1),
        "ba": f(ba).reshape(ACT, 1),
    }
    in_maps = []
    for c in range(8):
        b = c % B
        oh = np.zeros((N, 1), np.float32)
        oh[tgt[b], 0] = 1.0
        m = dict(shared)
        m["obsT"] = f(obs[b].T)
        m["onehot"] = oh
        in_maps.append(m)
    return in_maps


def run(in_maps, **kwargs):
    nc = get_nc()
    return bass_utils.run_bass_kernel_spmd(
        nc, in_maps, core_ids=list(range(8)), **kwargs)


def kernel(**inputs) -> np.ndarray:
    in_maps = make_in_maps(**inputs)
    res = run(in_maps)
    return np.stack(
        [res.results[b]["act_out"] for b in range(B)], axis=0).astype(np.float32)


# revision 4
# speedup vs baseline: 1.0747x; 1.0747x over previous
"""CoLightAgent forward kernel for 8 Trainium2 NeuronCores.

Math note: in the reference, ne = broadcast(adj @ emb) over the agent axis i,
so nh.sum(axis=3) / hid.sum(axis=3) are independent of i and collapse to
per-batch vectors S_n, S_h of shape [T].  The final gather keeps only row
tgt[b] of the agent branch.  The whole [B,N,N,T] intermediate disappears:

    E    = relu(relu(obs @ We1 + be1) @ We2 + be2)        # [N, T] per batch
    AE   = adj @ E                                        # [N, T]
    S_n  = sum_j relu(AE @ Wn + bn)[j, :]                 # [T]
    S_h  = sum_j relu(AE @ Wh + bh)[j, :]                 # [T]
    a    = relu(E[tgt] @ Wl + bl)                         # [T]
    attn = softmax_d((a * S_n).reshape(D, H).T)           # [H, D]
    g    = mean_h(attn * S_h.reshape(D, H).T)             # [D]
    act  = g @ Wa + ba                                    # [ACT]

Sharding: data-parallel over the batch; core c computes batch c % 4 in full
(cores 4..7 duplicate 0..3 and their outputs are ignored).  All matmuls keep
the contraction dim on partitions; activations flow as
    E1T [t, n] -> E [n, t'] -> AET [t, m]
so every stage feeds the next as lhsT/rhs without transposes.

The softmax runs entirely in the [T, 1] column layout: logits l = a*S_n are
>= 0 (product of relu outputs), so exp(min(l, 85)) never over/underflows and
matches the reference's max-subtracted softmax to fp32 accuracy.  Per-head
sums / broadcasts use tiny 0/1 selector matmuls (Gh, Gh^T, Gd) instead of a
DRAM round-trip for the (d h) -> h d regrouping.

Inputs are packed host-side into 4 DMAs issued from three different queues
(SP + ACT on HWDGE, Pool on SWDGE) so descriptor generation overlaps.
"""

import numpy as np

import concourse.bacc as bacc
import concourse.mybir as mybir
import concourse.tile as tile
from concourse import bass_utils
from concourse.bass import ts

B, N, OBS, ACT = 4, 256, 40, 8
HEAD, DIM = 8, 32
T = HEAD * DIM
P = 128
F32 = mybir.dt.float32
AF = mybir.ActivationFunctionType
AX = mybir.AxisListType
MULT = mybir.AluOpType.mult
CLAMP = 85.0

_CACHE = {}


def _build_nc():
    nc = bacc.Bacc("TRN2", target_bir_lowering=False, debug=False, num_devices=8)

    d_small = nc.dram_tensor("pk_small", [OBS, 2 * N], F32, kind="ExternalInput")
    d_bias = nc.dram_tensor("pk_bias", [N, 24], F32, kind="ExternalInput")
    d_a = nc.dram_tensor("pk_a", [5 * P, N], F32, kind="ExternalInput")
    d_b = nc.dram_tensor("pk_b", [8 * P, N], F32, kind="ExternalInput")
    d_out = nc.dram_tensor("act_out", [ACT], F32, kind="ExternalOutput")

    with tile.TileContext(nc) as tc:
        with (
            tc.tile_pool(name="w", bufs=1) as wp,
            tc.tile_pool(name="work", bufs=2) as work,
            tc.tile_pool(name="mmps", bufs=3, space="PSUM") as ps,
            tc.tile_pool(name="vecps", bufs=2, space="PSUM") as psv,
            tc.tile_pool(name="smps", bufs=1, space="PSUM") as pss,
        ):
            # ---- staged inputs --------------------------------------------
            small_t = wp.tile([P, 2 * N], F32)
            bias_t = wp.tile([P, 2, 24], F32)
            pa_t = wp.tile([P, 5, N], F32)
            pb_t = wp.tile([P, 8, N], F32)
            nc.any.memset(small_t[:], 0.0)
            nc.sync.dma_start(bias_t[:], d_bias.ap().rearrange("(s p) c -> p s c", p=P))
            nc.sync.dma_start(small_t[:OBS, :], d_small.ap())
            nc.scalar.dma_start(pa_t[:], d_a.ap().rearrange("(q p) m -> p q m", p=P))
            nc.gpsimd.dma_start(pb_t[:], d_b.ap().rearrange("(q p) m -> p q m", p=P))

            obsT = small_t[:, 0:N]
            We1 = small_t[:, N:2 * N]
            be1 = lambda s: bias_t[:, s, 0:1]
            bl = lambda s: bias_t[:, s, 1:2]
            bn = lambda s: bias_t[:, s, 2:3]
            bh = lambda s: bias_t[:, s, 3:4]
            oh = lambda s: bias_t[:, s, 4:5]
            ba = bias_t[0:ACT, 0, 5:6]
            Gh = lambda s: bias_t[:, s, 8:16]
            Wa = bias_t[:, 0, 16:24]
            We2 = lambda s: pa_t[:, s, :]
            adjT = lambda s: pa_t[:, 2 + s, :]
            be2f = pa_t[:, 4, :]
            Wn = lambda s: pb_t[:, s, :]
            Wh = lambda s: pb_t[:, 2 + s, :]
            Wl = lambda s: pb_t[:, 4 + s, :]
            GhT = pb_t[:, 6, :]
            Gd = lambda s: pb_t[:, 7, ts(s, DIM)]

            # ---- stage 1: E1T[t, n] = relu(We1.T @ obsT + be1) ------------
            E1T_t = wp.tile([P, 2, N], F32)
            for s in range(2):
                pm = ps.tile([P, N], F32, tag="mm")
                nc.tensor.matmul(pm[:], We1[:, ts(s, P)], obsT,
                                 start=True, stop=True)
                nc.scalar.activation(E1T_t[:, s, :], pm[:], AF.Relu, bias=be1(s))

            # ---- stage 2: E[n, t'] = relu(E1 @ We2 + be2) -----------------
            E_t = wp.tile([P, 2, T], F32)
            for s in range(2):
                pm = ps.tile([P, T], F32, tag="mm")
                nc.tensor.matmul(pm[:], E1T_t[:, 0, ts(s, P)], We2(0),
                                 start=True, stop=False)
                nc.tensor.matmul(pm[:], E1T_t[:, 1, ts(s, P)], We2(1),
                                 start=False, stop=True)
                nc.vector.tensor_add(E_t[:, s, :], pm[:], be2f)
                nc.vector.tensor_scalar_max(E_t[:, s, :], E_t[:, s, :], 0.0)

            # ---- stage 3: AET[t, m] = (adj @ E).T = E-as-lhsT @ adjT ------
            AET_t = wp.tile([P, 2, N], F32)
            for s in range(2):
                pm = ps.tile([P, N], F32, tag="mm")
                nc.tensor.matmul(pm[:], E_t[:, 0, ts(s, P)], adjT(0),
                                 start=True, stop=False)
                nc.tensor.matmul(pm[:], E_t[:, 1, ts(s, P)], adjT(1),
                                 start=False, stop=True)
                nc.vector.tensor_copy(AET_t[:, s, :], pm[:])

            # ---- stages 4/5: S_n / S_h (relu + row-sum fused) -------------
            Sn_t = wp.tile([P, 2, 1], F32)
            Sh_t = wp.tile([P, 2, 1], F32)
            for W, bv, S_t in ((Wn, bn, Sn_t), (Wh, bh, Sh_t)):
                for s in range(2):
                    pm = ps.tile([P, N], F32, tag="mm")
                    nc.tensor.matmul(pm[:], W(0)[:, ts(s, P)], AET_t[:, 0, :],
                                     start=True, stop=False)
                    nc.tensor.matmul(pm[:], W(1)[:, ts(s, P)], AET_t[:, 1, :],
                                     start=False, stop=True)
                    zt = work.tile([P, N], F32, tag="zsc")
                    nc.scalar.activation(zt[:], pm[:], AF.Relu, bias=bv(s),
                                         accum_out=S_t[:, s, :])

            # ---- stage 6: e_t = E[tgt, :] via one-hot ---------------------
            et_t = wp.tile([P, 2, 1], F32)
            for s in range(2):
                pv = psv.tile([P, 1], F32, tag="vec")
                nc.tensor.matmul(pv[:], E_t[:, 0, ts(s, P)], oh(0),
                                 start=True, stop=False)
                nc.tensor.matmul(pv[:], E_t[:, 1, ts(s, P)], oh(1),
                                 start=False, stop=True)
                nc.vector.tensor_copy(et_t[:, s, :], pv[:])

            # ---- stage 7: a = relu(Wl.T @ e_t + bl) -----------------------
            a_t = wp.tile([P, 2, 1], F32)
            for s in range(2):
                pv = psv.tile([P, 1], F32, tag="vec")
                nc.tensor.matmul(pv[:], Wl(0)[:, ts(s, P)], et_t[:, 0, :],
                                 start=True, stop=False)
                nc.tensor.matmul(pv[:], Wl(1)[:, ts(s, P)], et_t[:, 1, :],
                                 start=False, stop=True)
                nc.scalar.activation(a_t[:, s, :], pv[:], AF.Relu, bias=bl(s))

            # ---- softmax epilogue, all in [T, 1] column layout ------------
            # l = min(a * S_n, CLAMP); expl = exp(l)  (l >= 0 always)
            l_t = wp.tile([P, 2], F32)
            expl_t = wp.tile([P, 2], F32)
            nc.vector.tensor_mul(l_t[:], a_t[:, :, 0], Sn_t[:, :, 0])
            nc.vector.tensor_scalar_min(l_t[:], l_t[:], CLAMP)
            nc.scalar.activation(expl_t[:], l_t[:], AF.Exp)

            # denom[h] = sum_d expl[d*8+h];  recip = 1/denom
            recip_t = wp.tile([P, 1], F32)
            nc.any.memset(recip_t[:], 0.0)
            pd = pss.tile([HEAD, 1], F32, tag="tiny")
            nc.tensor.matmul(pd[:], Gh(0), expl_t[:, 0:1], start=True, stop=False)
            nc.tensor.matmul(pd[:], Gh(1), expl_t[:, 1:2], start=False, stop=True)
            nc.vector.reciprocal(recip_t[0:HEAD, :], pd[:])

            # v[t] = expl[t] * recip[t%8] * S_h[t]
            v_t = wp.tile([P, 2], F32)
            for s in range(2):
                pv = psv.tile([P, 1], F32, tag="vec")
                nc.tensor.matmul(pv[:], GhT[:, ts(s, P)], recip_t[:],
                                 start=True, stop=True)
                nc.vector.scalar_tensor_tensor(v_t[:, s:s + 1], expl_t[:, s:s + 1],
                                               pv[:], Sh_t[:, s, :], MULT, MULT)

            # g[d] = sum_h v[d*8+h];  act = (g @ Wa) / 8 + ba
            g_t = wp.tile([P, 1], F32)
            nc.any.memset(g_t[:], 0.0)
            pg = pss.tile([DIM, 1], F32, tag="tiny")
            nc.tensor.matmul(pg[:], Gd(0), v_t[:, 0:1], start=True, stop=False)
            nc.tensor.matmul(pg[:], Gd(1), v_t[:, 1:2], start=False, stop=True)
            nc.vector.tensor_copy(g_t[:DIM, :], pg[:])
            pa = pss.tile([ACT, 1], F32, tag="tiny")
            nc.tensor.matmul(pa[:], Wa, g_t[:], start=True, stop=True)
            res_t = wp.tile([ACT, 1], F32)
            nc.scalar.activation(res_t[:], pa[:], AF.Identity, bias=ba,
                                 scale=1.0 / HEAD)
            nc.sync.dma_start(d_out.ap(), res_t[:, 0])

    nc.compile()
    return nc


def get_nc():
    if "nc" not in _CACHE:
        _CACHE["nc"] = _build_nc()
    return _CACHE["nc"]


def _selectors():
    t = np.arange(T)
    Gh = (t[:, None] % HEAD == np.arange(HEAD)[None, :]).astype(np.float32)
    GhT_pad = np.zeros((P, T), np.float32)
    GhT_pad[:HEAD, :] = Gh.T
    GdSlab = np.zeros((P, N), np.float32)
    for s in range(2):
        tt = s * P + np.arange(P)
        GdSlab[:, s * DIM:(s + 1) * DIM] = (
            tt[:, None] // HEAD == np.arange(DIM)[None, :])
    return Gh, GhT_pad, GdSlab


def make_in_maps(x, adj, We1, be1, We2, be2, Wl, bl, Wn, bn, Wh, bh, Wa, ba):
    f = lambda v: np.ascontiguousarray(np.asarray(v, np.float32))
    x = f(x)
    tgt = x[:, -1, 0].astype(np.int32)
    obs = x[:, :-1, :]
    Gh, GhT_pad, GdSlab = _selectors()

    pk_bias = np.zeros((N, 24), np.float32)
    pk_bias[:, 0] = np.asarray(be1, np.float32)
    pk_bias[:, 1] = np.asarray(bl, np.float32)
    pk_bias[:, 2] = np.asarray(bn, np.float32)
    pk_bias[:, 3] = np.asarray(bh, np.float32)
    pk_bias[:ACT, 5] = np.asarray(ba, np.float32)
    pk_bias[:, 8:16] = Gh
    pk_bias[:DIM, 16:24] = np.asarray(Wa, np.float32)

    pk_a = np.concatenate(
        [f(We2), f(np.asarray(adj).T),
         np.broadcast_to(np.asarray(be2, np.float32), (P, T))], axis=0)
    pk_b = np.concatenate(
        [f(Wn), f(Wh), f(Wl), GhT_pad, GdSlab], axis=0)
    pk_a = np.ascontiguousarray(pk_a)
    pk_b = np.ascontiguousarray(pk_b)

    in_maps = []
    for c in range(8):
        b = c % B
        pb = pk_bias.copy()
        pb[tgt[b], 4] = 1.0
        in_maps.append({
            "pk_small": np.ascontiguousarray(
                np.concatenate([obs[b].T, f(We1)], axis=1)),
            "pk_bias": pb,
            "pk_a": pk_a,
            "pk_b": pk_b,
        })
    return in_maps


def run(in_maps, **kwargs):
    nc = get_nc()
    return bass_utils.run_bass_kernel_spmd(
        nc, in_maps, core_ids=list(range(8)), **kwargs)


def kernel(**inputs) -> np.ndarray:
    in_maps = make_in_maps(**inputs)
    res = run(in_maps)
    return np.stack(
        [res.results[b]["act_out"] for b in range(B)], axis=0).astype(np.float32)


# revision 8
# speedup vs baseline: 1.5856x; 1.4754x over previous
"""CoLightAgent forward kernel for 8 Trainium2 NeuronCores.

Math note: in the reference, ne = broadcast(adj @ emb) over the agent axis i,
so nh.sum(axis=3) / hid.sum(axis=3) are independent of i and collapse to
per-batch vectors S_n, S_h of shape [T].  The final gather keeps only row
tgt[b] of the agent branch.  The whole [B,N,N,T] intermediate disappears:

    E    = relu(relu(obs @ We1 + be1) @ We2 + be2)        # [N, T] per batch
    AE   = adj @ E                                        # [N, T]
    S_n  = sum_j relu(AE @ Wn + bn)[j, :]                 # [T]
    S_h  = sum_j relu(AE @ Wh + bh)[j, :]                 # [T]
    a    = relu(E[tgt] @ Wl + bl)                         # [T]
    attn = softmax_d((a * S_n).reshape(D, H).T)           # [H, D]
    g    = mean_h(attn * S_h.reshape(D, H).T)             # [D]
    act  = g @ Wa + ba                                    # [ACT]

Sharding: data-parallel over the batch; core c computes batch c % 4 in full
(cores 4..7 duplicate 0..3 and their outputs are ignored).  All matmuls keep
the contraction dim on partitions; activations flow as
    E1T [t, n] -> E [n, t'] -> AET [t, m]
so every stage feeds the next as lhsT/rhs without transposes.

The softmax runs entirely in the [T, 1] column layout: logits l = a*S_n are
>= 0 (product of relu outputs), so exp(min(l, 85)) never over/underflows and
matches the reference's max-subtracted softmax to fp32 accuracy.  Per-head
sums / broadcasts use tiny 0/1 selector matmuls (Gh, Gh^T, Gd) instead of a
DRAM round-trip for the (d h) -> h d regrouping.

Inputs are packed host-side into 4 DMAs issued from three different queues
(SP + ACT on HWDGE, Pool on SWDGE) so descriptor generation overlaps.
"""

import numpy as np

import concourse.bacc as bacc
import concourse.mybir as mybir
import concourse.tile as tile
from concourse import bass_utils
from concourse.bass import ts

B, N, OBS, ACT = 4, 256, 40, 8
HEAD, DIM = 8, 32
T = HEAD * DIM
P = 128
F32 = mybir.dt.float32
F32R = mybir.dt.float32r
AF = mybir.ActivationFunctionType
AX = mybir.AxisListType
MULT = mybir.AluOpType.mult
CLAMP = 85.0

_CACHE = {}


def _build_nc():
    nc = bacc.Bacc("TRN2", target_bir_lowering=False, debug=False, num_devices=8)

    d_small = nc.dram_tensor("pk_small", [OBS, 2 * N], F32, kind="ExternalInput")
    d_bias = nc.dram_tensor("pk_bias", [N, 24], F32, kind="ExternalInput")
    d_a1 = nc.dram_tensor("pk_a1", [3 * P, N], F32, kind="ExternalInput")
    d_a2 = nc.dram_tensor("pk_a2", [2 * P, N], F32, kind="ExternalInput")
    d_b1 = nc.dram_tensor("pk_b1", [4 * P, N], F32, kind="ExternalInput")
    d_b2 = nc.dram_tensor("pk_b2", [4 * P, N], F32, kind="ExternalInput")
    d_out = nc.dram_tensor("act_out", [ACT], F32, kind="ExternalOutput")

    with tile.TileContext(nc) as tc:
        with (
            tc.tile_pool(name="w", bufs=1) as wp,
            tc.tile_pool(name="work", bufs=2) as work,
            tc.tile_pool(name="mmps", bufs=3, space="PSUM") as ps,
            tc.tile_pool(name="vecps", bufs=2, space="PSUM") as psv,
            tc.tile_pool(name="smps", bufs=1, space="PSUM") as pss,
        ):
            # ---- staged inputs --------------------------------------------
            small_t = wp.tile([P, 2 * N], F32)
            bias_t = wp.tile([P, 2, 24], F32)
            pa1_t = wp.tile([P, 3, N], F32)
            pa2_t = wp.tile([P, 2, N], F32)
            pb1_t = wp.tile([P, 4, N], F32)
            pb2_t = wp.tile([P, 4, N], F32)
            nc.any.memset(small_t[:], 0.0)
            nc.sync.dma_start(small_t[:OBS, :], d_small.ap())
            nc.sync.dma_start(bias_t[:], d_bias.ap().rearrange("(s p) c -> p s c", p=P))
            nc.scalar.dma_start(pa1_t[:], d_a1.ap().rearrange("(q p) m -> p q m", p=P))
            nc.scalar.dma_start(pa2_t[:], d_a2.ap().rearrange("(q p) m -> p q m", p=P))
            nc.gpsimd.dma_start(pb1_t[:], d_b1.ap().rearrange("(q p) m -> p q m", p=P))
            nc.gpsimd.dma_start(pb2_t[:], d_b2.ap().rearrange("(q p) m -> p q m", p=P))

            obsT = small_t[:, 0:N]
            We1 = small_t[:, N:2 * N]
            be1 = lambda s: bias_t[:, s, 0:1]
            bl = lambda s: bias_t[:, s, 1:2]
            bn = lambda s: bias_t[:, s, 2:3]
            bh = lambda s: bias_t[:, s, 3:4]
            oh = lambda s: bias_t[:, s, 4:5]
            ba = bias_t[0:ACT, 0, 5:6]
            Gh = lambda s: bias_t[:, s, 8:16]
            Wa = bias_t[:, 0, 16:24]
            We2 = lambda s: pa1_t[:, s, :]
            be2f = pa1_t[:, 2, :]
            adjT = lambda s: pa2_t[:, s, :]
            Wn = lambda s: pb1_t[:, s, :]
            Wh = lambda s: pb1_t[:, 2 + s, :]
            Wl = lambda s: pb2_t[:, s, :]
            GhT = pb2_t[:, 2, :]
            Gd = lambda s: pb2_t[:, 3, ts(s, DIM)]

            # fp32r rounding passes for DMA-fed matmul operands (idle-engine
            # copies; the fp32r tensor-engine mode runs 4x faster than fp32)
            small_r = wp.tile([P, 2 * N], F32R)
            nc.vector.tensor_copy(small_r[:], small_t[:])
            obsT_r = small_r[:, 0:N]
            We1_r = small_r[:, N:2 * N]
            we2_r = wp.tile([P, 2, N], F32R)
            nc.gpsimd.tensor_copy(we2_r[:], pa1_t[:, 0:2, :])
            adjT_r = wp.tile([P, 2, N], F32R)
            nc.gpsimd.tensor_copy(adjT_r[:], pa2_t[:])
            pb1_r = wp.tile([P, 4, N], F32R)
            nc.gpsimd.tensor_copy(pb1_r[:], pb1_t[:])
            We2r = lambda s: we2_r[:, s, :]
            adjTr = lambda s: adjT_r[:, s, :]
            Wnr = lambda s: pb1_r[:, s, :]
            Whr = lambda s: pb1_r[:, 2 + s, :]

            # ---- stage 1: E1T[t, n] = relu(We1.T @ obsT + be1) ------------
            E1T_t = wp.tile([P, 2, N], F32R)
            for s in range(2):
                pm = ps.tile([P, N], F32, tag="mm")
                nc.tensor.matmul(pm[:], We1_r[:, ts(s, P)], obsT_r,
                                 start=True, stop=True)
                nc.scalar.activation(E1T_t[:, s, :], pm[:], AF.Relu, bias=be1(s))

            # ---- stage 2: E[n, t'] = relu(E1 @ We2 + be2) -----------------
            E_t = wp.tile([P, 2, T], F32R)
            for s in range(2):
                pm = ps.tile([P, T], F32, tag="mm")
                nc.tensor.matmul(pm[:], E1T_t[:, 0, ts(s, P)], We2r(0),
                                 start=True, stop=False)
                nc.tensor.matmul(pm[:], E1T_t[:, 1, ts(s, P)], We2r(1),
                                 start=False, stop=True)
                nc.vector.tensor_add(E_t[:, s, :], pm[:], be2f)
                nc.vector.tensor_scalar_max(E_t[:, s, :], E_t[:, s, :], 0.0)

            # ---- stage 3: AET[t, m] = (adj @ E).T = E-as-lhsT @ adjT ------
            AET_t = wp.tile([P, 2, N], F32R)
            for s in range(2):
                pm = ps.tile([P, N], F32, tag="mm")
                nc.tensor.matmul(pm[:], E_t[:, 0, ts(s, P)], adjTr(0),
                                 start=True, stop=False)
                nc.tensor.matmul(pm[:], E_t[:, 1, ts(s, P)], adjTr(1),
                                 start=False, stop=True)
                nc.vector.tensor_copy(AET_t[:, s, :], pm[:])

            # ---- stages 4/5: S_n / S_h (relu + row-sum fused) -------------
            Sn_t = wp.tile([P, 2, 1], F32)
            Sh_t = wp.tile([P, 2, 1], F32)
            for W, bv, S_t in ((Wnr, bn, Sn_t), (Whr, bh, Sh_t)):
                for s in range(2):
                    pm = ps.tile([P, N], F32, tag="mm")
                    nc.tensor.matmul(pm[:], W(0)[:, ts(s, P)], AET_t[:, 0, :],
                                     start=True, stop=False)
                    nc.tensor.matmul(pm[:], W(1)[:, ts(s, P)], AET_t[:, 1, :],
                                     start=False, stop=True)
                    zt = work.tile([P, N], F32, tag="zsc")
                    nc.scalar.activation(zt[:], pm[:], AF.Relu, bias=bv(s),
                                         accum_out=S_t[:, s, :])

            # ---- stage 6: e_t = E[tgt, :] via one-hot ---------------------
            et_t = wp.tile([P, 2, 1], F32)
            Ef = E_t[:].bitcast(F32)
            for s in range(2):
                pv = psv.tile([P, 1], F32, tag="vec")
                nc.tensor.matmul(pv[:], Ef[:, 0, ts(s, P)], oh(0),
                                 start=True, stop=False)
                nc.tensor.matmul(pv[:], Ef[:, 1, ts(s, P)], oh(1),
                                 start=False, stop=True)
                nc.vector.tensor_copy(et_t[:, s, :], pv[:])

            # ---- stage 7: a = relu(Wl.T @ e_t + bl) -----------------------
            a_t = wp.tile([P, 2, 1], F32)
            for s in range(2):
                pv = psv.tile([P, 1], F32, tag="vec")
                nc.tensor.matmul(pv[:], Wl(0)[:, ts(s, P)], et_t[:, 0, :],
                                 start=True, stop=False)
                nc.tensor.matmul(pv[:], Wl(1)[:, ts(s, P)], et_t[:, 1, :],
                                 start=False, stop=True)
                nc.scalar.activation(a_t[:, s, :], pv[:], AF.Relu, bias=bl(s))

            # ---- softmax epilogue, all in [T, 1] column layout ------------
            # l = min(a * S_n, CLAMP); expl = exp(l)  (l >= 0 always)
            l_t = wp.tile([P, 2], F32)
            expl_t = wp.tile([P, 2], F32)
            nc.vector.tensor_mul(l_t[:], a_t[:, :, 0], Sn_t[:, :, 0])
            nc.vector.tensor_scalar_min(l_t[:], l_t[:], CLAMP)
            nc.scalar.activation(expl_t[:], l_t[:], AF.Exp)

            # denom[h] = sum_d expl[d*8+h];  recip = 1/denom
            recip_t = wp.tile([P, 1], F32)
            nc.any.memset(recip_t[:], 0.0)
            pd = pss.tile([HEAD, 1], F32, tag="tiny")
            nc.tensor.matmul(pd[:], Gh(0), expl_t[:, 0:1], start=True, stop=False)
            nc.tensor.matmul(pd[:], Gh(1), expl_t[:, 1:2], start=False, stop=True)
            nc.vector.reciprocal(recip_t[0:HEAD, :], pd[:])

            # v[t] = expl[t] * recip[t%8] * S_h[t]
            v_t = wp.tile([P, 2], F32)
            for s in range(2):
                pv = psv.tile([P, 1], F32, tag="vec")
                nc.tensor.matmul(pv[:], GhT[:, ts(s, P)], recip_t[:],
                                 start=True, stop=True)
                nc.vector.scalar_tensor_tensor(v_t[:, s:s + 1], expl_t[:, s:s + 1],
                                               pv[:], Sh_t[:, s, :], MULT, MULT)

            # g[d] = sum_h v[d*8+h];  act = (g @ Wa) / 8 + ba
            g_t = wp.tile([P, 1], F32)
            nc.any.memset(g_t[:], 0.0)
            pg = pss.tile([DIM, 1], F32, tag="tiny")
            nc.tensor.matmul(pg[:], Gd(0), v_t[:, 0:1], start=True, stop=False)
            nc.tensor.matmul(pg[:], Gd(1), v_t[:, 1:2], start=False, stop=True)
            nc.vector.tensor_copy(g_t[:DIM, :], pg[:])
            pa = pss.tile([ACT, 1], F32, tag="tiny")
            nc.tensor.matmul(pa[:], Wa, g_t[:], start=True, stop=True)
            res_t = wp.tile([ACT, 1], F32)
            nc.scalar.activation(res_t[:], pa[:], AF.Identity, bias=ba,
                                 scale=1.0 / HEAD)
            nc.sync.dma_start(d_out.ap(), res_t[:, 0])

    nc.compile()
    return nc


def get_nc():
    if "nc" not in _CACHE:
        _CACHE["nc"] = _build_nc()
    return _CACHE["nc"]


def _selectors():
    t = np.arange(T)
    Gh = (t[:, None] % HEAD == np.arange(HEAD)[None, :]).astype(np.float32)
    GhT_pad = np.zeros((P, T), np.float32)
    GhT_pad[:HEAD, :] = Gh.T
    GdSlab = np.zeros((P, N), np.float32)
    for s in range(2):
        tt = s * P + np.arange(P)
        GdSlab[:, s * DIM:(s + 1) * DIM] = (
            tt[:, None] // HEAD == np.arange(DIM)[None, :])
    return Gh, GhT_pad, GdSlab


def make_in_maps(x, adj, We1, be1, We2, be2, Wl, bl, Wn, bn, Wh, bh, Wa, ba):
    f = lambda v: np.ascontiguousarray(np.asarray(v, np.float32))
    x = f(x)
    tgt = x[:, -1, 0].astype(np.int32)
    obs = x[:, :-1, :]
    Gh, GhT_pad, GdSlab = _selectors()

    pk_bias = np.zeros((N, 24), np.float32)
    pk_bias[:, 0] = np.asarray(be1, np.float32)
    pk_bias[:, 1] = np.asarray(bl, np.float32)
    pk_bias[:, 2] = np.asarray(bn, np.float32)
    pk_bias[:, 3] = np.asarray(bh, np.float32)
    pk_bias[:ACT, 5] = np.asarray(ba, np.float32)
    pk_bias[:, 8:16] = Gh
    pk_bias[:DIM, 16:24] = np.asarray(Wa, np.float32)

    pk_a1 = np.ascontiguousarray(np.concatenate(
        [f(We2), np.broadcast_to(np.asarray(be2, np.float32), (P, T))], axis=0))
    pk_a2 = np.ascontiguousarray(f(np.asarray(adj).T))
    pk_b1 = np.ascontiguousarray(np.concatenate([f(Wn), f(Wh)], axis=0))
    pk_b2 = np.ascontiguousarray(np.concatenate([f(Wl), GhT_pad, GdSlab], axis=0))

    in_maps = []
    for c in range(8):
        b = c % B
        pb = pk_bias.copy()
        pb[tgt[b], 4] = 1.0
        in_maps.append({
            "pk_small": np.ascontiguousarray(
                np.concatenate([obs[b].T, f(We1)], axis=1)),
            "pk_bias": pb,
            "pk_a1": pk_a1,
            "pk_a2": pk_a2,
            "pk_b1": pk_b1,
            "pk_b2": pk_b2,
        })
    return in_maps


def run(in_maps, **kwargs):
    nc = get_nc()
    return bass_utils.run_bass_kernel_spmd(
        nc, in_maps, core_ids=list(range(8)), **kwargs)


def kernel(**inputs) -> np.ndarray:
    in_maps = make_in_maps(**inputs)
    res = run(in_maps)
    return np.stack(
        [res.results[b]["act_out"] for b in range(B)], axis=0).astype(np.float32)


# revision 10
# speedup vs baseline: 1.7158x; 1.0821x over previous
"""CoLightAgent forward kernel for 8 Trainium2 NeuronCores.

Math note: in the reference, ne = broadcast(adj @ emb) over the agent axis i,
so nh.sum(axis=3) / hid.sum(axis=3) are independent of i and collapse to
per-batch vectors S_n, S_h of shape [T].  The final gather keeps only row
tgt[b] of the agent branch.  The whole [B,N,N,T] intermediate disappears:

    E    = relu(relu(obs @ We1 + be1) @ We2 + be2)        # [N, T] per batch
    AE   = adj @ E                                        # [N, T]
    S_n  = sum_j relu(AE @ Wn + bn)[j, :]                 # [T]
    S_h  = sum_j relu(AE @ Wh + bh)[j, :]                 # [T]
    a    = relu(E[tgt] @ Wl + bl)                         # [T]
    attn = softmax_d((a * S_n).reshape(D, H).T)           # [H, D]
    g    = mean_h(attn * S_h.reshape(D, H).T)             # [D]
    act  = g @ Wa + ba                                    # [ACT]

Sharding: data-parallel over the batch; core c computes batch c % 4 in full
(cores 4..7 duplicate 0..3 and their outputs are ignored).  All matmuls keep
the contraction dim on partitions; activations flow as
    E1T [t, n] -> E [n, t'] -> AET [t, m]
so every stage feeds the next as lhsT/rhs without transposes.

The softmax runs entirely in the [T, 1] column layout: logits l = a*S_n are
>= 0 (product of relu outputs), so exp(min(l, 85)) never over/underflows and
matches the reference's max-subtracted softmax to fp32 accuracy.  Per-head
sums / broadcasts use tiny 0/1 selector matmuls (Gh, Gh^T, Gd) instead of a
DRAM round-trip for the (d h) -> h d regrouping.

Inputs are packed host-side into 4 DMAs issued from three different queues
(SP + ACT on HWDGE, Pool on SWDGE) so descriptor generation overlaps.
"""

import numpy as np

import concourse.bacc as bacc
import concourse.mybir as mybir
import concourse.tile as tile
from concourse import bass_utils
from concourse.bass import ts

B, N, OBS, ACT = 4, 256, 40, 8
HEAD, DIM = 8, 32
T = HEAD * DIM
P = 128
F32 = mybir.dt.float32
F32R = mybir.dt.float32r
AF = mybir.ActivationFunctionType
AX = mybir.AxisListType
MULT = mybir.AluOpType.mult
CLAMP = 85.0

_CACHE = {}


def _build_nc():
    nc = bacc.Bacc("TRN2", target_bir_lowering=False, debug=False, num_devices=8)

    d_small = nc.dram_tensor("pk_small", [P, 2 * N + P], F32R, kind="ExternalInput")
    d_bias = nc.dram_tensor("pk_bias", [N, 24], F32, kind="ExternalInput")
    d_a1 = nc.dram_tensor("pk_a1", [3 * P, N], F32R, kind="ExternalInput")
    d_a2 = nc.dram_tensor("pk_a2", [2 * P, N], F32R, kind="ExternalInput")
    d_b1 = nc.dram_tensor("pk_b1", [4 * P, N], F32R, kind="ExternalInput")
    d_b2 = nc.dram_tensor("pk_b2", [4 * P, N], F32, kind="ExternalInput")
    d_out = nc.dram_tensor("act_out", [ACT], F32, kind="ExternalOutput")

    with tile.TileContext(nc) as tc:
        with (
            tc.tile_pool(name="w", bufs=1) as wp,
            tc.tile_pool(name="work", bufs=2) as work,
            tc.tile_pool(name="mmps", bufs=3, space="PSUM") as ps,
            tc.tile_pool(name="vecps", bufs=2, space="PSUM") as psv,
            tc.tile_pool(name="smps", bufs=1, space="PSUM") as pss,
        ):
            # ---- staged inputs --------------------------------------------
            small_t = wp.tile([P, 2 * N + P], F32R)
            bias_t = wp.tile([P, 2, 24], F32)
            pa1_t = wp.tile([P, 3, N], F32R)
            pa2_t = wp.tile([P, 2, N], F32R)
            pb1_t = wp.tile([P, 4, N], F32R)
            pb2_t = wp.tile([P, 4, N], F32)
            nc.sync.dma_start(small_t[:], d_small.ap())
            nc.sync.dma_start(bias_t[:], d_bias.ap().rearrange("(s p) c -> p s c", p=P))
            nc.scalar.dma_start(pa1_t[:], d_a1.ap().rearrange("(q p) m -> p q m", p=P))
            nc.scalar.dma_start(pa2_t[:], d_a2.ap().rearrange("(q p) m -> p q m", p=P))
            nc.gpsimd.dma_start(pb1_t[:], d_b1.ap().rearrange("(q p) m -> p q m", p=P))
            nc.gpsimd.dma_start(pb2_t[:], d_b2.ap().rearrange("(q p) m -> p q m", p=P))

            obsT = small_t[:, 0:N]
            We1 = small_t[:, N:2 * N]
            be1 = lambda s: bias_t[:, s, 0:1]
            bl = lambda s: bias_t[:, s, 1:2]
            bn = lambda s: bias_t[:, s, 2:3]
            bh = lambda s: bias_t[:, s, 3:4]
            oh = lambda s: bias_t[:, s, 4:5]
            ba = bias_t[0:ACT, 0, 5:6]
            Gh = lambda s: bias_t[:, s, 8:16]
            Wa = bias_t[:, 0, 16:24]
            Wl = lambda s: pb2_t[:, s, :]
            GhT = pb2_t[:, 2, :]
            Gd = lambda s: pb2_t[:, 3, ts(s, DIM)]

            obsT_r = small_t[:, 0:N]
            We1_r = small_t[:, N:2 * N]
            We2r = lambda s: pa1_t[:, s, :]
            adjTr = lambda s: pa2_t[:, s, :]
            Wnr = lambda s: pb1_t[:, s, :]
            Whr = lambda s: pb1_t[:, 2 + s, :]
            be2r = pa1_t[0:1, 2, :]
            ones1 = small_t[0:1, 2 * N:2 * N + P]

            # ---- stage 1: E1T[t, n] = relu(We1.T @ obsT + be1) ------------
            E1T_t = wp.tile([P, 2, N], F32R)
            for s in range(2):
                pm = ps.tile([P, N], F32, tag="mm")
                nc.tensor.matmul(pm[:], We1_r[:, ts(s, P)], obsT_r,
                                 start=True, stop=True)
                nc.scalar.activation(E1T_t[:, s, :], pm[:], AF.Relu, bias=be1(s))

            # ---- stage 2: E[n, t'] = relu(E1 @ We2 + be2) -----------------
            E_t = wp.tile([P, 2, T], F32R)
            for s in range(2):
                pm = ps.tile([P, T], F32, tag="mm")
                nc.tensor.matmul(pm[:], E1T_t[:, 0, ts(s, P)], We2r(0),
                                 start=True, stop=False)
                nc.tensor.matmul(pm[:], E1T_t[:, 1, ts(s, P)], We2r(1),
                                 start=False, stop=False)
                nc.tensor.matmul(pm[:], ones1, be2r, start=False, stop=True)
                nc.vector.tensor_scalar_max(E_t[:, s, :], pm[:], 0.0)

            # ---- stage 3: AET[t, m] = (adj @ E).T = E-as-lhsT @ adjT ------
            AET_t = wp.tile([P, 2, N], F32R)
            for s in range(2):
                pm = ps.tile([P, N], F32, tag="mm")
                nc.tensor.matmul(pm[:], E_t[:, 0, ts(s, P)], adjTr(0),
                                 start=True, stop=False)
                nc.tensor.matmul(pm[:], E_t[:, 1, ts(s, P)], adjTr(1),
                                 start=False, stop=True)
                nc.vector.tensor_copy(AET_t[:, s, :], pm[:])

            # ---- stages 4/5: S_n / S_h (relu + row-sum fused) -------------
            Sn_t = wp.tile([P, 2, 1], F32)
            Sh_t = wp.tile([P, 2, 1], F32)
            zeros_t = wp.tile([P, N], F32)
            nc.vector.memset(zeros_t[:], 0.0)
            for W, bv, S_t, eng in ((Wnr, bn, Sn_t, "act"), (Whr, bh, Sh_t, "dve")):
                for s in range(2):
                    pm = ps.tile([P, N], F32, tag="mm")
                    nc.tensor.matmul(pm[:], W(0)[:, ts(s, P)], AET_t[:, 0, :],
                                     start=True, stop=False)
                    nc.tensor.matmul(pm[:], W(1)[:, ts(s, P)], AET_t[:, 1, :],
                                     start=False, stop=True)
                    zt = work.tile([P, N], F32, tag="zsc")
                    if eng == "act":
                        nc.scalar.activation(zt[:], pm[:], AF.Relu, bias=bv(s),
                                             accum_out=S_t[:, s, :])
                    else:
                        nc.vector.scalar_tensor_tensor(
                            zt[:], pm[:], bv(s), zeros_t[:],
                            mybir.AluOpType.add, mybir.AluOpType.max,
                            accum_out=S_t[:, s, :])

            # ---- stage 6: e_t = E[tgt, :] via one-hot ---------------------
            et_t = wp.tile([P, 2, 1], F32)
            Ef = E_t[:].bitcast(F32)
            for s in range(2):
                pv = psv.tile([P, 1], F32, tag="vec")
                nc.tensor.matmul(pv[:], Ef[:, 0, ts(s, P)], oh(0),
                                 start=True, stop=False)
                nc.tensor.matmul(pv[:], Ef[:, 1, ts(s, P)], oh(1),
                                 start=False, stop=True)
                nc.vector.tensor_copy(et_t[:, s, :], pv[:])

            # ---- stage 7: a = relu(Wl.T @ e_t + bl) -----------------------
            a_t = wp.tile([P, 2, 1], F32)
            for s in range(2):
                pv = psv.tile([P, 1], F32, tag="vec")
                nc.tensor.matmul(pv[:], Wl(0)[:, ts(s, P)], et_t[:, 0, :],
                                 start=True, stop=False)
                nc.tensor.matmul(pv[:], Wl(1)[:, ts(s, P)], et_t[:, 1, :],
                                 start=False, stop=True)
                nc.scalar.activation(a_t[:, s, :], pv[:], AF.Relu, bias=bl(s))

            # ---- softmax epilogue, all in [T, 1] column layout ------------
            # l = min(a * S_n, CLAMP); expl = exp(l)  (l >= 0 always)
            l_t = wp.tile([P, 2], F32)
            expl_t = wp.tile([P, 2], F32)
            nc.vector.tensor_mul(l_t[:], a_t[:, :, 0], Sn_t[:, :, 0])
            nc.vector.tensor_scalar_min(l_t[:], l_t[:], CLAMP)
            nc.scalar.activation(expl_t[:], l_t[:], AF.Exp)

            # denom[h] = sum_d expl[d*8+h];  recip = 1/denom
            recip_t = wp.tile([P, 1], F32)
            nc.vector.memset(recip_t[:], 0.0)
            pd = pss.tile([HEAD, 1], F32, tag="tiny")
            nc.tensor.matmul(pd[:], Gh(0), expl_t[:, 0:1], start=True, stop=False)
            nc.tensor.matmul(pd[:], Gh(1), expl_t[:, 1:2], start=False, stop=True)
            nc.vector.reciprocal(recip_t[0:HEAD, :], pd[:])

            # v[t] = expl[t] * recip[t%8] * S_h[t]
            v_t = wp.tile([P, 2], F32)
            for s in range(2):
                pv = psv.tile([P, 1], F32, tag="vec")
                nc.tensor.matmul(pv[:], GhT[:, ts(s, P)], recip_t[:],
                                 start=True, stop=True)
                nc.vector.scalar_tensor_tensor(v_t[:, s:s + 1], expl_t[:, s:s + 1],
                                               pv[:], Sh_t[:, s, :], MULT, MULT)

            # g[d] = sum_h v[d*8+h];  act = (g @ Wa) / 8 + ba
            g_t = wp.tile([P, 1], F32)
            nc.vector.memset(g_t[:], 0.0)
            pg = pss.tile([DIM, 1], F32, tag="tiny")
            nc.tensor.matmul(pg[:], Gd(0), v_t[:, 0:1], start=True, stop=False)
            nc.tensor.matmul(pg[:], Gd(1), v_t[:, 1:2], start=False, stop=True)
            nc.vector.tensor_copy(g_t[:DIM, :], pg[:])
            pa = pss.tile([ACT, 1], F32, tag="tiny")
            nc.tensor.matmul(pa[:], Wa, g_t[:], start=True, stop=True)
            res_t = wp.tile([ACT, 1], F32)
            nc.scalar.activation(res_t[:], pa[:], AF.Identity, bias=ba,
                                 scale=1.0 / HEAD)
            nc.sync.dma_start(d_out.ap(), res_t[:, 0])

    nc.compile()
    return nc


def get_nc():
    if "nc" not in _CACHE:
        _CACHE["nc"] = _build_nc()
    return _CACHE["nc"]


def _selectors():
    t = np.arange(T)
    Gh = (t[:, None] % HEAD == np.arange(HEAD)[None, :]).astype(np.float32)
    GhT_pad = np.zeros((P, T), np.float32)
    GhT_pad[:HEAD, :] = Gh.T
    GdSlab = np.zeros((P, N), np.float32)
    for s in range(2):
        tt = s * P + np.arange(P)
        GdSlab[:, s * DIM:(s + 1) * DIM] = (
            tt[:, None] // HEAD == np.arange(DIM)[None, :])
    return Gh, GhT_pad, GdSlab


def make_in_maps(x, adj, We1, be1, We2, be2, Wl, bl, Wn, bn, Wh, bh, Wa, ba):
    f = lambda v: np.ascontiguousarray(np.asarray(v, np.float32))
    x = f(x)
    tgt = x[:, -1, 0].astype(np.int32)
    obs = x[:, :-1, :]
    Gh, GhT_pad, GdSlab = _selectors()

    pk_bias = np.zeros((N, 24), np.float32)
    pk_bias[:, 0] = np.asarray(be1, np.float32)
    pk_bias[:, 1] = np.asarray(bl, np.float32)
    pk_bias[:, 2] = np.asarray(bn, np.float32)
    pk_bias[:, 3] = np.asarray(bh, np.float32)
    pk_bias[:ACT, 5] = np.asarray(ba, np.float32)
    pk_bias[:, 8:16] = Gh
    pk_bias[:DIM, 16:24] = np.asarray(Wa, np.float32)

    slab2 = np.zeros((P, T), np.float32)
    slab2[0, :] = np.asarray(be2, np.float32)
    pk_a1 = np.ascontiguousarray(np.concatenate([f(We2), slab2], axis=0))
    pk_a2 = np.ascontiguousarray(f(np.asarray(adj).T))
    pk_b1 = np.ascontiguousarray(np.concatenate([f(Wn), f(Wh)], axis=0))
    pk_b2 = np.ascontiguousarray(np.concatenate([f(Wl), GhT_pad, GdSlab], axis=0))

    in_maps = []
    for c in range(8):
        b = c % B
        pb = pk_bias.copy()
        pb[tgt[b], 4] = 1.0
        in_maps.append({
            "pk_small": np.ascontiguousarray(np.concatenate(
                [np.pad(obs[b].T, ((0, P - OBS), (0, 0))),
                 np.pad(f(We1), ((0, P - OBS), (0, 0))),
                 np.eye(1, P * 1, 0, np.float32).T @ np.ones((1, P), np.float32)
                 ], axis=1)),
            "pk_bias": pb,
            "pk_a1": pk_a1,
            "pk_a2": pk_a2,
            "pk_b1": pk_b1,
            "pk_b2": pk_b2,
        })
    return in_maps


def run(in_maps, **kwargs):
    nc = get_nc()
    return bass_utils.run_bass_kernel_spmd(
        nc, in_maps, core_ids=list(range(8)), **kwargs)


def kernel(**inputs) -> np.ndarray:
    in_maps = make_in_maps(**inputs)
    res = run(in_maps)
    return np.stack(
        [res.results[b]["act_out"] for b in range(B)], axis=0).astype(np.float32)


# revision 18
# speedup vs baseline: 2.0532x; 1.1967x over previous
"""CoLightAgent forward kernel for 8 Trainium2 NeuronCores.

Math note: in the reference, ne = broadcast(adj @ emb) over the agent axis i,
so nh.sum(axis=3) / hid.sum(axis=3) are independent of i and collapse to
per-batch vectors S_n, S_h of shape [T].  The final gather keeps only row
tgt[b] of the agent branch.  The whole [B,N,N,T] intermediate disappears:

    E    = relu(relu(obs @ We1 + be1) @ We2 + be2)        # [N, T] per batch
    AE   = adj @ E                                        # [N, T]
    S_n  = sum_j relu(AE @ Wn + bn)[j, :]                 # [T]
    S_h  = sum_j relu(AE @ Wh + bh)[j, :]                 # [T]
    a    = relu(E[tgt] @ Wl + bl)                         # [T]
    attn = softmax_d((a * S_n).reshape(D, H).T)           # [H, D]
    g    = mean_h(attn * S_h.reshape(D, H).T)             # [D]
    act  = g @ Wa + ba                                    # [ACT]

Sharding: data-parallel over the batch; core c computes batch c % 4 in full
(cores 4..7 duplicate 0..3 and their outputs are ignored).  All matmuls keep
the contraction dim on partitions; activations flow as
    E1T [t, n] -> E [n, t'] -> AET [t, m]
so every stage feeds the next as lhsT/rhs without transposes.

The softmax runs entirely in the [T, 1] column layout: logits l = a*S_n are
>= 0 (product of relu outputs), so exp(min(l, 85)) never over/underflows and
matches the reference's max-subtracted softmax to fp32 accuracy.  Per-head
sums / broadcasts use tiny 0/1 selector matmuls (Gh, Gh^T, Gd) instead of a
DRAM round-trip for the (d h) -> h d regrouping.

Inputs are packed host-side into 4 DMAs issued from three different queues
(SP + ACT on HWDGE, Pool on SWDGE) so descriptor generation overlaps.
"""

import numpy as np

import concourse.bacc as bacc
import concourse.mybir as mybir
import concourse.tile as tile
from concourse import bass_utils
from concourse.bass import ts

B, N, OBS, ACT = 4, 256, 40, 8
HEAD, DIM = 8, 32
T = HEAD * DIM
P = 128
F32 = mybir.dt.float32
F32R = mybir.dt.float32r
AF = mybir.ActivationFunctionType
AX = mybir.AxisListType
MULT = mybir.AluOpType.mult
CLAMP = 85.0

_CACHE = {}


def _build_nc():
    nc = bacc.Bacc("TRN2", target_bir_lowering=False, debug=False, num_devices=8)

    d_small = nc.dram_tensor("pk_small", [P, 2 * N + 48], F32R, kind="ExternalInput")
    d_tiny = nc.dram_tensor("pk_tiny", [1, 384], F32R, kind="ExternalInput")
    d_we2 = nc.dram_tensor("pk_we2", [2 * P, N], F32R, kind="ExternalInput")
    d_adjt = nc.dram_tensor("pk_adjt", [2 * P, N], F32R, kind="ExternalInput")
    d_wn = nc.dram_tensor("pk_wn", [2 * P, N], F32R, kind="ExternalInput")
    d_wl = nc.dram_tensor("pk_wl", [2 * P, N], F32, kind="ExternalInput")
    d_wh = nc.dram_tensor("pk_wh", [2 * P, N], F32R, kind="ExternalInput")
    d_sel = nc.dram_tensor("pk_sel", [2 * P, N], F32, kind="ExternalInput")
    d_out = nc.dram_tensor("act_out", [ACT], F32, kind="ExternalOutput")

    with tile.TileContext(nc) as tc:
        with (
            tc.tile_pool(name="w", bufs=1) as wp,
            tc.tile_pool(name="work", bufs=2) as work,
            tc.tile_pool(name="mmps", bufs=4, space="PSUM") as ps,
            tc.tile_pool(name="vecps", bufs=2, space="PSUM") as psv,
            tc.tile_pool(name="smps", bufs=1, space="PSUM") as pss,
        ):
            # ---- staged inputs --------------------------------------------
            small_t = wp.tile([P, 2 * N + 48], F32R)
            tiny_t = wp.tile([1, 384], F32R)
            we2_t = wp.tile([P, 2, N], F32R)
            adjt_t = wp.tile([P, 2, N], F32R)
            wn_t = wp.tile([P, 2, N], F32R)
            wl_t = wp.tile([P, 2, N], F32)
            wh_t = wp.tile([P, 2, N], F32R)
            sel_t = wp.tile([P, 2, N], F32)
            rq = lambda d: d.ap().rearrange("(q p) m -> p q m", p=P)
            nc.sync.dma_start(small_t[:], d_small.ap())
            nc.scalar.dma_start(we2_t[:], rq(d_we2))
            nc.sync.dma_start(tiny_t[:], d_tiny.ap())
            nc.scalar.dma_start(adjt_t[:], rq(d_adjt))
            nc.sync.dma_start(wn_t[:], rq(d_wn))
            nc.scalar.dma_start(wl_t[:], rq(d_wl))
            nc.sync.dma_start(wh_t[:], rq(d_wh))
            nc.sync.dma_start(sel_t[:], rq(d_sel))
            bias_t = small_t[:, 2 * N:2 * N + 48].rearrange(
                "p (s c) -> p s c", c=24).bitcast(F32)

            obsT = small_t[:, 0:N]
            We1 = small_t[:, N:2 * N]
            be1 = lambda s: bias_t[:, s, 0:1]
            bl = lambda s: bias_t[:, s, 1:2]
            bn = lambda s: bias_t[:, s, 2:3]
            bh = lambda s: bias_t[:, s, 3:4]
            oh = lambda s: bias_t[:, s, 4:5]
            ba = bias_t[0:ACT, 0, 5:6]
            Gh = lambda s: bias_t[:, s, 8:16]
            Wa = bias_t[:, 0, 16:24]
            Wl = lambda s: wl_t[:, s, :]
            GhT = sel_t[:, 0, :]
            Wbig = lambda s: sel_t[:, 1, ts(s, ACT)]

            obsT_r = small_t[:, 0:N]
            We1_r = small_t[:, N:2 * N]
            We2r = lambda s: we2_t[:, s, :]
            adjTr = lambda s: adjt_t[:, s, :]
            Wnr = lambda s: wn_t[:, s, :]
            Whr = lambda s: wh_t[:, s, :]
            ones1 = tiny_t[0:1, 0:P]
            be2r = tiny_t[0:1, P:P + N]

            zeros_t = wp.tile([P, N], F32)
            nc.vector.memset(zeros_t[:], 0.0)

            # ---- stage 1: E1T[t, n] = relu(We1.T @ obsT + be1) ------------
            E1T_t = wp.tile([P, 2, N], F32R)
            for s in range(2):
                pm = ps.tile([P, N], F32, tag="mm")
                nc.tensor.matmul(pm[:], We1_r[:, ts(s, P)], obsT_r,
                                 start=True, stop=True)
                if s == 0:
                    nc.scalar.activation(E1T_t[:, s, :], pm[:], AF.Relu,
                                         bias=be1(s))
                else:
                    nc.vector.scalar_tensor_tensor(
                        E1T_t[:, s, :], pm[:], be1(s), zeros_t[:],
                        mybir.AluOpType.add, mybir.AluOpType.max)

            # ---- stage 2: E[n, t'] = relu(E1 @ We2 + be2) -----------------
            E_t = wp.tile([P, 2, T], F32R)
            for s in range(2):
                pm = ps.tile([P, T], F32, tag="mm")
                nc.tensor.matmul(pm[:], E1T_t[:, 0, ts(s, P)], We2r(0),
                                 start=True, stop=False)
                nc.tensor.matmul(pm[:], E1T_t[:, 1, ts(s, P)], We2r(1),
                                 start=False, stop=False)
                nc.tensor.matmul(pm[:], ones1, be2r, start=False, stop=True)
                if s == 0:
                    nc.vector.tensor_scalar_max(E_t[:, s, :], pm[:], 0.0)
                else:
                    nc.scalar.activation(E_t[:, s, :], pm[:], AF.Relu)

            # ---- stage 3: AET[t, m] = (adj @ E).T = E-as-lhsT @ adjT ------
            AET_t = wp.tile([P, 2, N], F32R)
            for s in range(2):
                pm = ps.tile([P, N], F32, tag="mm")
                nc.tensor.matmul(pm[:], E_t[:, 0, ts(s, P)], adjTr(0),
                                 start=True, stop=False)
                nc.tensor.matmul(pm[:], E_t[:, 1, ts(s, P)], adjTr(1),
                                 start=False, stop=True)
                nc.vector.tensor_copy(AET_t[:, s, :], pm[:])

            # ---- stages 4/5: S_n / S_h (relu + row-sum fused) -------------
            # S_n gates the softmax chain, so its two slices run in parallel
            # on ACT and DVE; S_h (only needed later by v) follows.
            Sn_t = wp.tile([P, 2, 1], F32)
            Sh_t = wp.tile([P, 2, 1], F32)

            def relu_rowsum(W, bv, S_t):
                for s in range(2):
                    pm = ps.tile([P, N], F32, tag="mm")
                    nc.tensor.matmul(pm[:], W(0)[:, ts(s, P)], AET_t[:, 0, :],
                                     start=True, stop=False)
                    nc.tensor.matmul(pm[:], W(1)[:, ts(s, P)], AET_t[:, 1, :],
                                     start=False, stop=True)
                    zt = work.tile([P, N], F32, tag="zsc")
                    if s == 0:
                        nc.scalar.activation(zt[:], pm[:], AF.Relu, bias=bv(s),
                                             accum_out=S_t[:, s, :])
                    else:
                        nc.vector.scalar_tensor_tensor(
                            zt[:], pm[:], bv(s), zeros_t[:],
                            mybir.AluOpType.add, mybir.AluOpType.max,
                            accum_out=S_t[:, s, :])

            relu_rowsum(Wnr, bn, Sn_t)

            # ---- stage 6: e_t = E[tgt, :] via one-hot ---------------------
            et_t = wp.tile([P, 2, 1], F32)
            Ef = E_t[:].bitcast(F32)
            for s in range(2):
                pv = psv.tile([P, 1], F32, tag="vec")
                nc.tensor.matmul(pv[:], Ef[:, 0, ts(s, P)], oh(0),
                                 start=True, stop=False)
                nc.tensor.matmul(pv[:], Ef[:, 1, ts(s, P)], oh(1),
                                 start=False, stop=True)
                nc.vector.tensor_copy(et_t[:, s, :], pv[:])

            # ---- stage 7: a = relu(Wl.T @ e_t + bl) -----------------------
            a_t = wp.tile([P, 2, 1], F32)
            for s in range(2):
                pv = psv.tile([P, 1], F32, tag="vec")
                nc.tensor.matmul(pv[:], Wl(0)[:, ts(s, P)], et_t[:, 0, :],
                                 start=True, stop=False)
                nc.tensor.matmul(pv[:], Wl(1)[:, ts(s, P)], et_t[:, 1, :],
                                 start=False, stop=True)
                nc.scalar.activation(a_t[:, s, :], pv[:], AF.Relu, bias=bl(s))

            # ---- softmax epilogue, all in [T, 1] column layout ------------
            # l = min(a * S_n, CLAMP); expl = exp(l)  (l >= 0 always)
            l_t = wp.tile([P, 2], F32)
            expl_t = wp.tile([P, 2], F32)
            nc.vector.tensor_mul(l_t[:], a_t[:, :, 0], Sn_t[:, :, 0])
            nc.vector.tensor_scalar_min(l_t[:], l_t[:], CLAMP)
            nc.scalar.activation(expl_t[:], l_t[:], AF.Exp)

            with tc.high_priority(offset=-300):
                relu_rowsum(Whr, bh, Sh_t)

            # denom[h] = sum_d expl[d*8+h];  recip = 1/denom
            recip_t = wp.tile([P, 1], F32)
            nc.vector.memset(recip_t[:], 0.0)
            pd = pss.tile([HEAD, 1], F32, tag="tiny")
            nc.tensor.matmul(pd[:], Gh(0), expl_t[:, 0:1], start=True, stop=False)
            nc.tensor.matmul(pd[:], Gh(1), expl_t[:, 1:2], start=False, stop=True)
            nc.vector.reciprocal(recip_t[0:HEAD, :], pd[:])

            # v[t] = expl[t] * recip[t%8] * S_h[t]
            # act[a] = sum_t Wbig[t, a] * v[t]   (Wbig[t,a] = Wa[t//8,a]/8)
            v_t = wp.tile([P, 2], F32)
            pa = pss.tile([ACT, 1], F32, tag="tiny")
            for s in range(2):
                pv = psv.tile([P, 1], F32, tag="vec")
                nc.tensor.matmul(pv[:], GhT[:, ts(s, P)], recip_t[:],
                                 start=True, stop=True)
                nc.vector.scalar_tensor_tensor(v_t[:, s:s + 1], expl_t[:, s:s + 1],
                                               pv[:], Sh_t[:, s, :], MULT, MULT)
            nc.tensor.matmul(pa[:], Wbig(0), v_t[:, 0:1], start=True, stop=False)
            nc.tensor.matmul(pa[:], Wbig(1), v_t[:, 1:2], start=False, stop=True)
            res_t = wp.tile([ACT, 1], F32)
            nc.scalar.activation(res_t[:], pa[:], AF.Identity, bias=ba, scale=1.0)
            nc.sync.dma_start(d_out.ap(), res_t[:, 0])

    nc.compile()
    return nc


def get_nc():
    if "nc" not in _CACHE:
        _CACHE["nc"] = _build_nc()
    return _CACHE["nc"]


def _selectors():
    t = np.arange(T)
    Gh = (t[:, None] % HEAD == np.arange(HEAD)[None, :]).astype(np.float32)
    GhT_pad = np.zeros((P, T), np.float32)
    GhT_pad[:HEAD, :] = Gh.T
    return Gh, GhT_pad


def make_in_maps(x, adj, We1, be1, We2, be2, Wl, bl, Wn, bn, Wh, bh, Wa, ba):
    f = lambda v: np.ascontiguousarray(np.asarray(v, np.float32))
    x = f(x)
    tgt = x[:, -1, 0].astype(np.int32)
    obs = x[:, :-1, :]
    Gh, GhT_pad = _selectors()

    pk_bias = np.zeros((N, 24), np.float32)
    pk_bias[:, 0] = np.asarray(be1, np.float32)
    pk_bias[:, 1] = np.asarray(bl, np.float32)
    pk_bias[:, 2] = np.asarray(bn, np.float32)
    pk_bias[:, 3] = np.asarray(bh, np.float32)
    pk_bias[:ACT, 5] = np.asarray(ba, np.float32)
    pk_bias[:, 8:16] = Gh
    pk_bias[:DIM, 16:24] = np.asarray(Wa, np.float32)


    pk_tiny = np.zeros((1, 384), np.float32)
    pk_tiny[0, 0:P] = 1.0
    pk_tiny[0, P:P + N] = np.asarray(be2, np.float32)
    pk_we2 = f(We2)
    pk_adjt = np.ascontiguousarray(f(np.asarray(adj).T))
    pk_wn = f(Wn)
    pk_wl = f(Wl)
    pk_wh = f(Wh)
    WbigSlab = np.zeros((P, N), np.float32)
    Wa8 = np.asarray(Wa, np.float32) / HEAD
    for si in range(2):
        tt = si * P + np.arange(P)
        WbigSlab[:, si * ACT:(si + 1) * ACT] = Wa8[tt // HEAD, :]
    pk_sel = np.ascontiguousarray(np.concatenate([GhT_pad, WbigSlab], axis=0))

    in_maps = []
    for c in range(8):
        b = c % B
        pb = pk_bias.copy()
        pb[tgt[b], 4] = 1.0
        in_maps.append({
            "pk_small": np.ascontiguousarray(np.concatenate(
                [np.pad(obs[b].T, ((0, P - OBS), (0, 0))),
                 np.pad(f(We1), ((0, P - OBS), (0, 0))),
                 pb.reshape(2, P, 24).transpose(1, 0, 2).reshape(P, 48),
                 ], axis=1)),
            "pk_tiny": pk_tiny,
            "pk_we2": pk_we2,
            "pk_adjt": pk_adjt,
            "pk_wn": pk_wn,
            "pk_wl": pk_wl,
            "pk_wh": pk_wh,
            "pk_sel": pk_sel,
        })
    return in_maps


def run(in_maps, **kwargs):
    nc = get_nc()
    return bass_utils.run_bass_kernel_spmd(
        nc, in_maps, core_ids=list(range(8)), **kwargs)


def kernel(**inputs) -> np.ndarray:
    in_maps = make_in_maps(**inputs)
    res = run(in_maps)
    return np.stack(
        [res.results[b]["act_out"] for b in range(B)], axis=0).astype(np.float32)
